# revision 19
# baseline (speedup 1.0000x reference)
"""AtomicCharge Trainium2 kernel (nn_AtomicCharge_77781857730661), v3.

Strategy
--------
Data-parallel over atoms across 8 NeuronCores. Exact-fill packing:
atoms map to slots 1:1 (slot = atom index) over a 1024-row x T grid
(T=1984, keeping DMA lines 64B-aligned), molecules SPLIT at row
boundaries; the device emits two fixup values per row (V[:,T-1] and
RL at forward col 0) from which the host corrects the <=1023 split
molecules vectorized. No greedy packing.

x is uploaded pre-transposed bf16 in j-major stream order. aux is 3
bf16 planes (pCH=chg/n, pIV=1/n at segment starts, mA=start mask) + 1
zero column; the reverse-scan mask is derived on device as a
shifted-reversed view of mA (saves the 4th plane vs baseline).

Per core (raw bass, explicit semaphores):
  PE:  per group (4 chunks x NB atoms): 4x mm1 (W1^T x, bf16)
       col-tiled 2x on the PE array; 2x K=128 mm2 with the w2p
       placement stationary accumulating atom_out into the packed
       [128,NB] panel PSUM. mm2 lags 6 groups in steady state and
       CATCHES UP over groups 116-119 so only groups 126/127 of mm2
       remain after the last mm1 (short drain).
  ACT: one [128,2CW] Silu per group (bias b1 fused, bf16 out).
  DVE: IV expansion scan; per block j: panel drain (+b2, bf16),
       aoiv=ao*IV, d1=pCH-aoiv, V-scan (CH-expansion and FL-scan fused
       into ONE scan: V_c = mA_c*V_{c-1} + d1_c), Q=V+aoiv+ao.
       Epilogue: 2-chunk reverse scan of aoiv with the shifted-mA
       mask, quarters out = Q - rev(RL) DMA'd as computed. All
       epilogue tensors bf16 (2x DVE elementwise throughput). No
       intra-DVE semaphore self-waits (engine is in-order).
  GPSIMD: w2p zero-fill.

HW: ~190 us/core on trn2 (baseline 203-206); rel err ~5e-3 (bf16).
"""
import sys

sys.path.insert(0, "/opt/trn_rl_repo")

import numpy as np
import ml_dtypes

import concourse.bass as bass
from concourse import mybir
from concourse.bass_utils import run_bass_kernel_spmd
import concourse.bass_utils as _bu

# Enable walrus's redundant-LDWEIGHTS elimination (off by default in this
# stack); our mm1s reuse the same stationaries within a group.
if not getattr(_bu, "_ldwopt_patched", False):
    _orig_run_command = _bu.run_command

    def _run_command_ldwopt(argv, **kw):
        argv = [a.replace("--enable-ldw-opt=false", "--enable-ldw-opt=true")
                for a in argv]
        return _orig_run_command(argv, **kw)

    _bu.run_command = _run_command_ldwopt
    _bu._ldwopt_patched = True

F32 = mybir.dt.float32
BF16 = mybir.dt.bfloat16
NP_BF16 = ml_dtypes.bfloat16

# problem constants (hardcoded per spec)
N_ATOMS = 2_000_000
N_MOL = 50_000
D = 128      # node feature dim = SBUF partitions
H = 64       # hidden dim
NCORES = 8
R = 128      # atom-layout rows per core (partitions)
T = 1984     # slots per row; mult of 32 so DMA lines stay 64B-aligned

_NC_CACHE = {}
LAST_RUN_INFO = {}


def build_raw(T):
    NB = T // 4
    NPAIR = R // 2
    NBLK = 4
    NG = NPAIR // 2             # 32 groups per block; group = 4 rows x NB
    S = R * T
    CW = 512                    # chunk stride inside hp/hs (bank-aligned)
    WW = 2 * CW
    GW = 4 * NB                 # xT columns per group
    XPG = 4                     # groups per x slab
    NXP = 5                     # x slab buffers
    NHS = 10                    # hs buffers
    MM2_LAG = 6
    T2 = T // 2
    TQ = T // 4
    AOp = mybir.AluOpType

    nc = bass.Bass()
    xT = nc.declare_dram_parameter("xT", [D, S], BF16, isOutput=False)
    W1 = nc.declare_dram_parameter("W1", [D, H], BF16, isOutput=False)
    b1s = nc.declare_dram_parameter("b1s", [D], F32, isOutput=False)
    b2 = nc.declare_dram_parameter("b2", [1], F32, isOutput=False)
    W2s = nc.declare_dram_parameter("W2s", [D, 1], F32, isOutput=False)
    aux = nc.declare_dram_parameter("aux", [R, 3 * T + 1], BF16, isOutput=False)
    out = nc.declare_dram_parameter("out", [R, T + 2], BF16, isOutput=True)

    from contextlib import ExitStack
    with ExitStack() as ctx:
        def sbuf(shape, dtype, name):
            return ctx.enter_context(nc.sbuf_tensor(name, shape, dtype))

        def psum(shape, name):
            return ctx.enter_context(nc.psum_tensor(name, shape, F32))

        w1a = sbuf([D, H], BF16, "w1a")
        w1b = sbuf([D, H], BF16, "w1b")
        b1t = sbuf([D, 1], F32, "b1t")
        b2t = sbuf([D, 1], F32, "b2t")
        w2s = sbuf([D, 1], F32, "w2s")
        w2p = sbuf([D, NPAIR * D], BF16, "w2p")
        auxt = sbuf([R, 3 * T + 1], BF16, "auxt")
        xp = [sbuf([D, XPG * GW], BF16, f"xp{s}") for s in range(NXP)]
        hs = [sbuf([D, WW], BF16, f"hs{s}") for s in range(NHS)]
        aoS = sbuf([R, NB], BF16, "aoS")       # per-block drain scratch
        d1S = sbuf([R, NB], BF16, "d1S")
        IV = sbuf([R, T], BF16, "IV")
        V = sbuf([R, T], BF16, "V")
        aoiv = sbuf([R, T], BF16, "aoiv")
        Q = sbuf([R, T], BF16, "Q")
        RLr = sbuf([R, T], BF16, "RLr")
        obuf = sbuf([R, T + 2], BF16, "obuf")

        hp = [psum([D, WW], f"hp{s}") for s in range(3)]     # 2 banks each
        panels = [psum([R, NB], f"panel{s}") for s in range(2)]

        s_w = ctx.enter_context(nc.semaphore("s_w"))
        s_wz = ctx.enter_context(nc.semaphore("s_wz"))
        s_w2p = ctx.enter_context(nc.semaphore("s_w2p"))
        s_aux = ctx.enter_context(nc.semaphore("s_aux"))
        s_x = [ctx.enter_context(nc.semaphore(f"s_x{i}")) for i in range(NXP)]
        s_mm1 = ctx.enter_context(nc.semaphore("s_mm1"))
        s_hs = ctx.enter_context(nc.semaphore("s_hs"))
        s_mm2 = ctx.enter_context(nc.semaphore("s_mm2"))
        s_pan = ctx.enter_context(nc.semaphore("s_pan"))
        s_dve = ctx.enter_context(nc.semaphore("s_dve"))
        s_ep = ctx.enter_context(nc.semaphore("s_ep"))
        s_out = ctx.enter_context(nc.semaphore("s_out"))
        block = ctx.enter_context(nc.Block())

        pCHt = auxt[:, 0 * T:1 * T]
        pIVt = auxt[:, 1 * T:2 * T]
        mAt = auxt[:, 2 * T:3 * T]
        # zero column at auxt col 3T (host supplies 0)

        def rev(ap, n=None, end=None):
            """Reverse view over the free dim: elements end-1, ..., end-n."""
            n = T if n is None else n
            end = T if end is None else end
            return bass.AP(tensor=ap.tensor, offset=ap.offset + (end - 1),
                           ap=[list(ap.ap[0]), [-1, n]])

        def mrev(nrev, crev0):
            """Reverse-scan mask chunk: element i (global reversed col
            crev0+i) = mA[T - (crev0+i)], with mA[T] := the zero col."""
            base = auxt[:, 3 * T - crev0:3 * T - crev0 + 1]
            return bass.AP(tensor=base.tensor, offset=base.offset,
                           ap=[list(base.ap[0]), [-1, nrev]])

        NGT = NBLK * NG          # 128
        SLABS = [(4 * i, 4) for i in range(30)] + [(120 + i, 1) for i in range(8)]
        slab_of_group = {}
        for i, (st, nsl) in enumerate(SLABS):
            for k in range(nsl):
                slab_of_group[st + k] = i

        # mm2 emission schedule: lag 6 steady, catch up to lag 2 at the end
        EX = {g: [] for g in range(NGT)}
        gp = 0
        for g in range(MM2_LAG, 116):
            EX[g].append(gp); gp += 1            # 0..109
        for g in range(116, 120):
            EX[g] += [gp, gp + 1]; gp += 2       # 110..117
        for g in range(120, 128):
            EX[g].append(gp); gp += 1            # 118..125
        TAIL_MM2 = list(range(gp, NGT))          # 126, 127
        assert gp == 126

        # ---------------- SP: all DMA traffic ----------------
        @block.sync
        def _(sync):
            def xdma(i):
                st, nsl = SLABS[i]
                sync.dma_start(out=xp[i % NXP][:, 0:nsl * GW],
                               in_=xT[:, st * GW:(st + nsl) * GW]
                               ).then_inc(s_x[i % NXP], 16)

            xdma(0)
            sync.dma_start(out=w1a[:], in_=W1[:]).then_inc(s_w, 16)
            sync.dma_start(out=w1b[:], in_=W1[:]).then_inc(s_w, 16)
            sync.dma_start(out=b1t[:], in_=b1s[:, None]).then_inc(s_w, 16)
            b2bc = bass.AP(tensor=b2.ap().tensor, offset=0, ap=[[0, D], [1, 1]])
            sync.dma_start(out=b2t[:], in_=b2bc).then_inc(s_w, 16)
            sync.dma_start(out=w2s[:], in_=W2s[:]).then_inc(s_w, 16)
            for i in range(1, NXP):
                xdma(i)
            for i in range(NXP, len(SLABS)):
                pst, pn = SLABS[i - NXP]
                sync.wait_ge(s_mm1, pst + pn)
                xdma(i)
                if i == NXP:
                    sync.dma_start(out=auxt[:, 0:2 * T],
                                   in_=aux[:, 0:2 * T]).then_inc(s_aux, 16)
                elif i == NXP + 1:
                    sync.dma_start(out=auxt[:, 2 * T:3 * T + 1],
                                   in_=aux[:, 2 * T:3 * T + 1]
                                   ).then_inc(s_aux, 16)
            # output quarters q3..q1 (q0 is issued by the vector engine;
            # it includes the 2 fixup columns at obuf[:,0:2])
            sync.wait_ge(s_ep, 1)
            sync.dma_start(out=out[:, 2 + 3 * TQ:2 + T],
                           in_=obuf[:, 2 + 3 * TQ:2 + T]).then_inc(s_out, 16)
            sync.wait_ge(s_ep, 2)
            sync.dma_start(out=out[:, 2 + 2 * TQ:2 + 3 * TQ],
                           in_=obuf[:, 2 + 2 * TQ:2 + 3 * TQ]
                           ).then_inc(s_out, 16)
            sync.wait_ge(s_ep, 3)
            sync.dma_start(out=out[:, 2 + TQ:2 + 2 * TQ],
                           in_=obuf[:, 2 + TQ:2 + 2 * TQ]).then_inc(s_out, 16)
            sync.wait_ge(s_out, 64)

        # ---------------- PE ----------------
        @block.tensor
        def _(tensor):
            tensor.wait_ge(s_w, 80)
            first_mm2 = [True]

            def mm2_pair(gp):
                jp, ggp = divmod(gp, NG)
                if first_mm2[0]:
                    tensor.wait_ge(s_w2p, 1)
                    first_mm2[0] = False
                tensor.wait_ge(s_hs, gp + 1)
                if ggp == 0 and jp >= 2:
                    tensor.wait_ge(s_pan, jp - 1)
                for c in range(2):
                    kp = 2 * ggp + c
                    nc.tensor.matmul(
                        out=panels[jp % 2][:],
                        lhsT=w2p[:, kp * D:(kp + 1) * D],
                        rhs=hs[gp % NHS][:, c * CW:c * CW + NB],
                        start=(ggp == 0 and c == 0),
                        stop=(ggp == NG - 1 and c == 1)).then_inc(s_mm2, 1)

            for g in range(NGT):
                i = slab_of_group[g]
                st, nsl = SLABS[i]
                if g == st:
                    tensor.wait_ge(s_x[i % NXP], 16 * (i // NXP + 1))
                # WAR: hp[g%3] reused -> silu(g-3) must be done
                if g >= 3:
                    tensor.wait_ge(s_hs, g - 2)
                xbase = (g - st) * GW
                xslot = xp[i % NXP]
                last = None
                for c in range(4):
                    po = 64 * (c & 1)
                    col = CW * (c >> 1)
                    last = nc.tensor.matmul(
                        out=hp[g % 3][po:po + 64, col:col + NB],
                        lhsT=(w1a if po == 0 else w1b)[:],
                        rhs=xslot[:, xbase + c * NB:xbase + (c + 1) * NB],
                        start=True, stop=True,
                        tile_position=(0, po))
                last.then_inc(s_mm1, 1)
                for gp in EX[g]:
                    mm2_pair(gp)
            for gp in TAIL_MM2:
                mm2_pair(gp)

        # ---------------- ACT: one [128,2CW] silu per group -------------
        @block.scalar
        def _(scalar):
            scalar.wait_ge(s_w, 80)
            for g in range(NGT):
                scalar.wait_ge(s_mm1, g + 1)
                # WAR: hs[g%NHS] reused -> mm2(g-NHS) must be done
                if g >= NHS:
                    scalar.wait_ge(s_mm2, 2 * (g - NHS + 1))
                nc.scalar.activation(
                    out=hs[g % NHS][:], in_=hp[g % 3][:],
                    func=mybir.ActivationFunctionType.Silu,
                    bias=b1t[:], scale=1.0,
                ).then_inc(s_hs, 1)
            # final output quarter (incl fixup cols) issued here: scalar
            # is idle at the tail while sync is busy issuing q1
            scalar.wait_ge(s_ep, 4)
            scalar.dma_start(out=out[:, 0:2 + TQ],
                             in_=obuf[:, 0:2 + TQ]).then_inc(s_out, 16)

        # ---------------- GPSIMD: w2p zero-fill ------------------------
        @block.gpsimd
        def _(gp_):
            nc.gpsimd.memset(w2p[:], 0.0).then_inc(s_wz, 1)

        # ---------------- DVE: w2p fill, drains + incremental epilogue --
        # NOTE: the DVE pipelines instructions without hazard interlock;
        # every consecutive dependent pair (RAW on the previous op's
        # output) needs an explicit semaphore self-wait (step) or results
        # are intermittently corrupted.
        @block.vector
        def _(vector):
            tick = [0]

            def step(ins):
                ins.then_inc(s_dve, 1)
                tick[0] += 1
                vector.wait_ge(s_dve, tick[0])

            # build w2p placement stationaries: block k has W2 lo at col
            # 130k (partitions 0:64) and W2 hi at col 130k+1 (64:128)
            vector.wait_ge(s_w, 80)
            vector.wait_ge(s_wz, 1)
            base_lo = w2p[0:64, 0:1]
            view_lo = bass.AP(tensor=base_lo.tensor, offset=base_lo.offset,
                              ap=[list(base_lo.ap[0]), [130, NPAIR]])
            base_hi = w2p[64:128, 1:2]
            view_hi = bass.AP(tensor=base_hi.tensor, offset=base_hi.offset,
                              ap=[list(base_hi.ap[0]), [130, NPAIR]])
            nc.vector.tensor_scalar_add(view_lo, view_lo, w2s[0:64, 0:1])
            nc.vector.tensor_scalar_add(
                view_hi, view_hi, w2s[64:128, 0:1]).then_inc(s_w2p, 1)

            vector.wait_ge(s_aux, 32)
            # IV = segmented expansion of pIV
            nc.vector.tensor_tensor_scan(
                out=IV[:], data0=mAt, data1=pIVt,
                initial=0.0, op0=AOp.mult, op1=AOp.add)

            for j in range(NBLK):
                lo, hi = j * NB, (j + 1) * NB
                vector.wait_ge(s_mm2, 2 * NG * (j + 1))
                nc.vector.tensor_scalar_add(
                    aoS[:], panels[j % 2][:], b2t[:]).then_inc(s_pan, 1)
                vector.wait_ge(s_pan, j + 1)            # aoS ready
                step(nc.vector.tensor_mul(
                    aoiv[:, lo:hi], aoS[:], IV[:, lo:hi]))
                step(nc.vector.tensor_sub(d1S[:], pCHt[:, lo:hi],
                                          aoiv[:, lo:hi]))
                init = 0.0 if j == 0 else V[:, lo - 1:lo]
                step(nc.vector.tensor_tensor_scan(
                    out=V[:, lo:hi], data0=mAt[:, lo:hi], data1=d1S[:],
                    initial=init, op0=AOp.mult, op1=AOp.add))
                if j == NBLK - 1:
                    nc.vector.tensor_copy(obuf[:, 0:1], V[:, T - 1:T])
                step(nc.vector.tensor_add(Q[:, lo:hi], V[:, lo:hi],
                                          aoiv[:, lo:hi]))
                step(nc.vector.tensor_add(Q[:, lo:hi], Q[:, lo:hi], aoS[:]))

            # tail: 2-chunk reverse scan of aoiv with shifted-mA mask;
            # out = Q - rev(RL), emitted in quarters
            step(nc.vector.tensor_tensor_scan(
                out=RLr[:, 0:T2], data0=mrev(T2, 0), data1=rev(aoiv[:], n=T2),
                initial=0.0, op0=AOp.mult, op1=AOp.add))
            nc.vector.tensor_sub(
                obuf[:, 2 + 3 * TQ:2 + T], Q[:, 3 * TQ:T],
                rev(RLr[:], n=TQ, end=TQ)).then_inc(s_ep, 1)
            nc.vector.tensor_sub(
                obuf[:, 2 + 2 * TQ:2 + 3 * TQ], Q[:, 2 * TQ:3 * TQ],
                rev(RLr[:], n=TQ, end=2 * TQ)).then_inc(s_ep, 1)
            step(nc.vector.tensor_tensor_scan(
                out=RLr[:, T2:T], data0=mrev(T2, T2),
                data1=rev(aoiv[:], n=T2, end=T2),
                initial=RLr[:, T2 - 1:T2], op0=AOp.mult, op1=AOp.add))
            nc.vector.tensor_sub(
                obuf[:, 2 + TQ:2 + 2 * TQ], Q[:, TQ:2 * TQ],
                rev(RLr[:], n=TQ, end=3 * TQ)).then_inc(s_ep, 1)
            nc.vector.tensor_copy(obuf[:, 1:2], RLr[:, T - 1:T])
            nc.vector.tensor_sub(
                obuf[:, 2:2 + TQ], Q[:, 0:TQ],
                rev(RLr[:], n=TQ, end=T)).then_inc(s_ep, 1)

    return nc


def build_nc(T):
    if T in _NC_CACHE:
        return _NC_CACHE[T]
    nc = build_raw(T)
    _NC_CACHE[T] = nc
    return nc


def kernel(x_scalar, batch, charge, W1, b1, W2, b2):
    x_scalar = np.asarray(x_scalar, dtype=np.float32)
    batch = np.asarray(batch, dtype=np.int32)
    charge = np.asarray(charge, dtype=np.float32)
    W1 = np.asarray(W1, dtype=np.float32)
    b1 = np.asarray(b1, dtype=np.float32)
    W2 = np.asarray(W2, dtype=np.float32)
    b2 = np.asarray(b2, dtype=np.float32)
    n = x_scalar.shape[0]

    # tolerate unsorted batch (reference data is sorted; insurance)
    order = None
    if np.any(np.diff(batch) < 0):
        order = np.argsort(batch, kind="stable")
        x_scalar = x_scalar[order]
        batch = batch[order]

    NB = T // 4
    NPAIR = R // 2
    NBLK = 4
    S = R * T
    NROWS = NCORES * R
    SLOTS = NROWS * T
    n_mol = charge.shape[0]
    assert n <= SLOTS, "atoms do not fit the exact-fill grid"
    cnt = np.bincount(batch, minlength=n_mol).astype(np.int64)
    assert cnt.max() <= T, "molecule larger than a row"

    # ---- masks (exact fill: slot = atom index) ----
    b64 = batch.astype(np.int64)
    starts = np.concatenate([[0], np.flatnonzero(np.diff(b64)) + 1])
    m_st = batch[starts]
    mA = np.ones(SLOTS, np.float32)
    pCH = np.zeros(SLOTS, np.float32)
    pIV = np.zeros(SLOTS, np.float32)
    mA[starts] = 0.0
    pCH[starts] = (charge[m_st] / cnt[m_st]).astype(np.float32)
    pIV[starts] = (1.0 / cnt[m_st]).astype(np.float32)
    rs = np.arange(NROWS, dtype=np.int64) * T
    rs_v = rs[rs < n]
    m_rs = batch[rs_v]
    mA[rs_v] = 0.0
    pCH[rs_v] = (charge[m_rs] / cnt[m_rs]).astype(np.float32)
    pIV[rs_v] = (1.0 / cnt[m_rs]).astype(np.float32)
    mA[n:] = 0.0
    splits = rs_v[(rs_v > 0) & (batch[np.maximum(rs_v - 1, 0)] == batch[rs_v])]

    mA = mA.reshape(NROWS, T)
    pCH = pCH.reshape(NROWS, T)
    pIV = pIV.reshape(NROWS, T)

    # ---- packed transposed x per core (bf16, j-major stream order) ----
    xpad = np.zeros((NCORES * S, D), NP_BF16)
    xpad[:n] = x_scalar.astype(NP_BF16)
    xT_cores = []
    for c in range(NCORES):
        a = xpad[c * S:(c + 1) * S].reshape(NPAIR, 2, NBLK, NB, D)
        a = a.transpose(2, 0, 1, 3, 4).reshape(S, D)
        xT_cores.append(np.ascontiguousarray(a.T))
    del xpad

    W2s = np.concatenate([W2[:, 0], W2[:, 0]]).reshape(D, 1).astype(np.float32)
    W1b = W1.astype(NP_BF16)
    b1s = np.concatenate([b1, b1]).astype(np.float32)

    nc = build_nc(T)
    in_maps = []
    zcol = np.zeros((R, 1), np.float32)
    for c in range(NCORES):
        sl = slice(c * R, (c + 1) * R)
        auxc = np.concatenate([pCH[sl], pIV[sl], mA[sl], zcol],
                              axis=1).astype(NP_BF16)
        in_maps.append({
            "xT": xT_cores[c], "W1": W1b, "b1s": b1s, "b2": b2, "W2s": W2s,
            "aux": np.ascontiguousarray(auxc),
        })

    import os
    trace = bool(int(os.environ.get("ATOMIC_TRACE", "0")))
    res = run_bass_kernel_spmd(nc, in_maps, list(range(NCORES)), trace=trace)
    LAST_RUN_INFO["exec_time_ns"] = getattr(res, "exec_time_ns", None)
    LAST_RUN_INFO["profile_json"] = getattr(res, "profile_json", None)

    outs = [res.results[c]["out"] for c in range(NCORES)]
    Vtail = np.concatenate([o[:, 0].astype(np.float32) for o in outs])
    RLtail = np.concatenate([o[:, 1].astype(np.float32) for o in outs])
    big = np.concatenate([o[:, 2:].astype(np.float32).reshape(-1)
                          for o in outs])
    at = big[:n].copy()

    # ---- split-molecule fixup ----
    if len(splits):
        m = batch[splits]
        rB = splits // T
        rA = rB - 1
        sumB_n = RLtail[rB]
        sumA_n = (charge[m] / cnt[m]).astype(np.float32) - Vtail[rA]
        sA = np.searchsorted(b64, m.astype(np.int64), 'left')
        eB = np.searchsorted(b64, m.astype(np.int64), 'right')
        corr = np.zeros(n + 1, np.float32)
        # part A [sA, split): subtract sumB/n ; part B [split, eB): sumA/n
        np.add.at(corr, sA, sumB_n)
        np.add.at(corr, splits, -sumB_n + sumA_n)
        np.add.at(corr, eB, -sumA_n)
        at -= np.cumsum(corr[:-1])

    if order is not None:
        inv = np.empty_like(order)
        inv[order] = np.arange(n)
        at = at[inv]
    return at
